# revision 8
# baseline (speedup 1.0000x reference)
"""Distributed Trainium2 Bass kernel for NeuTRENO attention (B=2, N=2048, C=1024, H=16).

Sharding: 8 cores = (batch b in {0,1}) x (head-group hg in {0..3}, 4 heads each).
Cores [4b, 4b+3] share batch b; the output projection partial sums are
ReduceScattered within each 4-core group.

Per core:
  phase 1: qT,kT (feature-major) and v (token-major) projections from xT.
  phase 2 per head:
    chain B: scores^T [m,n] tiles -> exp (ScalarE, bf16) -> attn@v on PE
             (contraction over m on partitions), accumulated over m-chunks.
    chain A: scores [n,m] tiles -> exp with accumulator rowsums ->
             reciprocal -> per-partition normalize (DVE) -> 1MB DMAs of attn.
  phase 2.5: PE-transpose av^T -> av [n,d], normalizing by recip during the
             PSUM eviction; NeuTRENO residual alpha*(v0 - v) added.
  phase 3: output projection partials x_outT = Wproj_sliceT @ out_T,
           ReduceScatter(add) within the 4-core group, + bias, DMA out.

Outputs per core: attn shard [4, 2048, 2048] f32 and x_outT shard [256, 2048] f32.
Host assembles the full (x_out, attn) tuple.
"""

import os
import sys
from contextlib import ExitStack

import numpy as np
import ml_dtypes

for _p in ("/opt/trn_rl_repo", "/root/.axon_site/_ro/trn_rl_repo"):
    if os.path.isdir(_p) and _p not in sys.path:
        sys.path.append(_p)

import concourse.bass as bass
import concourse.tile as tile
from concourse import bacc, mybir
from concourse.bass import ds, ts
from concourse.bass_utils import run_bass_kernel_spmd
from concourse.masks import make_identity

F32 = mybir.dt.float32
BF16 = mybir.dt.bfloat16
BF16NP = ml_dtypes.bfloat16

B, N, C, H = 2, 2048, 1024, 16
Dh = C // H          # 64
HPC = 4              # heads per core
NCORES = 8
SCALE = Dh ** -0.5
ALPHA = 0.6

NT = N // 128        # 16 token tiles of 128
CC = C // 128        # 8 contraction chunks
QKF = 2 * HPC * Dh   # 512 q+k features per core
VF = HPC * Dh        # 256 v features per core


KPHASE = int(os.environ.get("KPHASE", "4"))  # 1=proj, 2=+attn, 3=+transp/resid, 4=full


def build_nc():
    nc = bacc.Bacc(
        "TRN2", target_bir_lowering=False, debug=False, num_devices=NCORES
    )

    # ---- per-core DRAM parameters (host supplies shards) ----
    xT_d = nc.dram_tensor("xT", [CC, 128, N], BF16, kind="ExternalInput").ap()
    wqkT_d = nc.dram_tensor("wqkT", [CC, 128, QKF], BF16, kind="ExternalInput").ap()
    bqk_d = nc.dram_tensor("bqk", [QKF // 128, 128], F32, kind="ExternalInput").ap()
    wvT_d = nc.dram_tensor("wvT", [CC, 128, VF], BF16, kind="ExternalInput").ap()
    bv_d = nc.dram_tensor("bvb", [128, VF], F32, kind="ExternalInput").ap()
    av0_d = nc.dram_tensor("av0", [NT, 128, VF], F32, kind="ExternalInput").ap()
    wpT_d = nc.dram_tensor("wpT", [VF // 128, 128, C], BF16, kind="ExternalInput").ap()
    bp_d = nc.dram_tensor("bp", [2, 128], F32, kind="ExternalInput").ap()

    attn_d = nc.dram_tensor("attn", [HPC, N, N], F32, kind="ExternalOutput").ap()
    xout_d = nc.dram_tensor("xout", [VF, N], F32, kind="ExternalOutput").ap()

    with tile.TileContext(nc) as tc, ExitStack() as ctx:
        persist = ctx.enter_context(tc.tile_pool(name="persist", bufs=1))
        dram = ctx.enter_context(tc.tile_pool(name="dram", bufs=1, space="DRAM"))

        # persistent SBUF tensors
        qkT_s = persist.tile([128, QKF // 128, N], BF16)   # q,k feature-major
        v_s = persist.tile([128, NT, VF], F32)             # v token-major (residual)
        v_bf = persist.tile([128, NT, VF], BF16)           # v token-major (av lhsT)
        av0_s = persist.tile([128, NT, VF], F32)           # alpha * v0, token-major
        out_res = persist.tile([128, NT, VF], F32)         # out = av*recip + a(v0-v)
        bv_s = persist.tile([128, VF], F32)
        bqk_s = persist.tile([128, QKF // 128], F32)
        bp_s = persist.tile([128, 2], F32)
        ident = persist.tile([128, 128], F32)
        recip_s = persist.tile([128, HPC, NT], F32)        # 1/rowsum per head+ntile
        avT_s = persist.tile([64, HPC, N], F32)            # raw av^T per head

        make_identity(nc, ident)

        nc.sync.dma_start(out=bv_s, in_=bv_d)
        for fc in range(QKF // 128):
            nc.sync.dma_start(out=bqk_s[:, fc : fc + 1], in_=bqk_d[fc].unsqueeze(1))
        for i in range(2):
            nc.sync.dma_start(out=bp_s[:, i : i + 1], in_=bp_d[i].unsqueeze(1))
        nc.sync.dma_start(
            out=av0_s, in_=av0_d.rearrange("nt p d -> p nt d")
        )

        # ---------------- phase 1: projections ----------------
        with tc.tile_pool(name="p1sb", bufs=2) as p1sb, \
             tc.tile_pool(name="p1ps", bufs=2, space="PSUM") as p1ps, \
             tc.tile_pool(name="p1psv", bufs=2, space="PSUM") as p1psv:
            xT_s = persist.tile([128, CC, N], BF16)
            wqk_s = persist.tile([128, CC, QKF], BF16)
            wv_s = persist.tile([128, CC, VF], BF16)
            for cc in range(CC):
                nc.sync.dma_start(out=xT_s[:, cc, :], in_=xT_d[cc])
                nc.sync.dma_start(out=wqk_s[:, cc, :], in_=wqkT_d[cc])
                nc.sync.dma_start(out=wv_s[:, cc, :], in_=wvT_d[cc])

            # qT,kT: [128 f, N] tiles, contraction over c
            for fc in range(QKF // 128):
                qk_ps = p1ps.tile([128, 1024], F32, tag="qk")
                qk_ps2 = p1ps.tile([128, 1024], F32, tag="qk")
                for half, pst in ((0, qk_ps), (1, qk_ps2)):
                    for n4 in range(2):
                        for cc in range(CC):
                            nc.tensor.matmul(
                                pst[:, ts(n4, 512)],
                                lhsT=wqk_s[:, cc, ts(fc, 128)],
                                rhs=xT_s[:, cc, ds(half * 1024 + n4 * 512, 512)],
                                start=(cc == 0),
                                stop=(cc == CC - 1),
                            )
                    nc.scalar.activation(
                        out=qkT_s[:, fc, ds(half * 1024, 1024)],
                        in_=pst,
                        func=mybir.ActivationFunctionType.Identity,
                        bias=bqk_s[:, fc : fc + 1],
                    )

            # v token-major: [128 n, VF] tiles
            for nt in range(NT):
                v_ps = p1psv.tile([128, VF], F32, tag="v")
                for cc in range(CC):
                    nc.tensor.matmul(
                        v_ps,
                        lhsT=xT_s[:, cc, ts(nt, 128)],
                        rhs=wv_s[:, cc, :],
                        start=(cc == 0),
                        stop=(cc == CC - 1),
                    )
                nc.vector.tensor_tensor(
                    out=v_s[:, nt, :], in0=v_ps, in1=bv_s,
                    op=mybir.AluOpType.add,
                )
                nc.vector.tensor_tensor(
                    out=v_bf[:, nt, :], in0=v_ps, in1=bv_s,
                    op=mybir.AluOpType.add,
                )

        # ---------------- phase 2: attention per head ----------------
        with tc.tile_pool(name="p2sb", bufs=3) as p2sb, \
             tc.tile_pool(name="p2at", bufs=3) as p2at, \
             tc.tile_pool(name="p2z", bufs=4) as p2z, \
             tc.tile_pool(name="psA", bufs=1, space="PSUM") as psA, \
             tc.tile_pool(name="psB", bufs=2, space="PSUM") as psB:
            for h in range(HPC if KPHASE >= 2 else 0):
                bp0 = (h % 2) * 64  # base partition of this head's q/k rows
                fq, fk = h // 2, 2 + h // 2
                qT = qkT_s[bp0 : bp0 + 64, fq, :]
                kT = qkT_s[bp0 : bp0 + 64, fk, :]

                # ---- chain B: scores^T -> exp -> av accumulation ----
                av_ps = psA.tile([64, N], F32, tag="av")
                for mt in range(NT):
                    esT = p2sb.tile([128, N], BF16, tag="esT")
                    for half in range(2):
                        st_ps = psB.tile([128, 1024], F32, tag="big")
                        for n4 in range(2):
                            nc.tensor.matmul(
                                st_ps[:, ts(n4, 512)],
                                lhsT=kT[:, ts(mt, 128)],
                                rhs=qT[:, ds(half * 1024 + n4 * 512, 512)],
                                start=True,
                                stop=True,
                            )
                        nc.scalar.activation(
                            out=esT[:, ds(half * 1024, 1024)],
                            in_=st_ps,
                            func=mybir.ActivationFunctionType.Exp,
                        )
                    for n4 in range(4):
                        nc.tensor.matmul(
                            av_ps[:, ts(n4, 512)],
                            lhsT=v_bf[:, mt, ds(h * Dh, Dh)],
                            rhs=esT[:, ts(n4, 512)],
                            start=(mt == 0),
                            stop=(mt == NT - 1),
                        )
                nc.vector.tensor_copy(out=avT_s[:, h, :], in_=av_ps)

                # ---- chain A: scores -> exp(+rowsum) -> normalize -> DMA ----
                for nt in range(NT):
                    at = p2at.tile([128, N], F32, tag="attn")
                    za = p2z.tile([128, 1], F32, tag="z")
                    zb = p2z.tile([128, 1], F32, tag="z")
                    for half, zz in ((0, za), (1, zb)):
                        s_ps = psB.tile([128, 1024], F32, tag="big")
                        for m4 in range(2):
                            nc.tensor.matmul(
                                s_ps[:, ts(m4, 512)],
                                lhsT=qT[:, ts(nt, 128)],
                                rhs=kT[:, ds(half * 1024 + m4 * 512, 512)],
                                start=True,
                                stop=True,
                            )
                        nc.scalar.activation(
                            out=at[:, ds(half * 1024, 1024)],
                            in_=s_ps,
                            func=mybir.ActivationFunctionType.Exp,
                            accum_out=zz,
                        )
                    nc.vector.tensor_tensor(
                        out=za, in0=za, in1=zb, op=mybir.AluOpType.add
                    )
                    nc.vector.reciprocal(
                        out=recip_s[:, h, nt : nt + 1], in_=za
                    )
                    nc.vector.tensor_scalar_mul(
                        out=at, in0=at, scalar1=recip_s[:, h, nt : nt + 1]
                    )
                    nc.sync.dma_start(out=attn_d[h, ts(nt, 128), :], in_=at)

        # ------------- phase 2.5: transpose av^T, normalize, residual -------------
        with tc.tile_pool(name="psT", bufs=2, space="PSUM") as psT:
            for h in range(HPC if KPHASE >= 3 else 0):
                for nt in range(NT):
                    tr_ps = psT.tile([128, 64], F32, tag="tr")
                    nc.tensor.transpose(
                        tr_ps, avT_s[:, h, ts(nt, 128)], ident[:64, :64]
                    )
                    nc.vector.tensor_scalar_mul(
                        out=out_res[:, nt, ds(h * Dh, Dh)],
                        in0=tr_ps,
                        scalar1=recip_s[:, h, nt : nt + 1],
                    )
            # residual: out_res += alpha*v0 - alpha*v   (av0_s is pre-scaled)
            with tc.tile_pool(name="resid", bufs=3) as resid:
                for nt in range(NT if KPHASE >= 3 else 0):
                    t_r = resid.tile([128, VF], F32, tag="t_r")
                    nc.vector.tensor_scalar_mul(
                        out=t_r, in0=v_s[:, nt, :], scalar1=ALPHA
                    )
                    nc.vector.tensor_tensor(
                        out=t_r, in0=av0_s[:, nt, :], in1=t_r,
                        op=mybir.AluOpType.subtract,
                    )
                    nc.vector.tensor_tensor(
                        out=out_res[:, nt, :], in0=out_res[:, nt, :], in1=t_r,
                        op=mybir.AluOpType.add,
                    )

        # ---------------- phase 3: output projection + ReduceScatter ----------------
        xoutT_d = dram.tile([C, N], F32)
        rs_d = dram.tile([VF, N], F32)
        with tc.tile_pool(name="p3sb", bufs=2) as p3sb, \
             tc.tile_pool(name="p3ps", bufs=2, space="PSUM") as p3ps, \
             tc.tile_pool(name="p3tr", bufs=2, space="PSUM") as p3tr:
            if KPHASE < 4:
                dummy = p3sb.tile([128, 8], F32, tag="dummy")
                nc.vector.memset(dummy, 0.0)
                raise_phase = True
            wp_s = p3sb.tile([128, VF // 128, C], BF16, tag="wp")
            for cc in range(VF // 128 if KPHASE >= 4 else 0):
                nc.sync.dma_start(out=wp_s[:, cc, :], in_=wpT_d[cc])
            # transpose out_res -> out_T [c, n] (bf16)
            outT_s = p3sb.tile([128, VF // 128, N], BF16, tag="outT")
            for nt in range(NT if KPHASE >= 4 else 0):
                for cc in range(VF // 128):
                    tr_ps = p3tr.tile([128, 128], F32, tag="tr3")
                    nc.tensor.transpose(
                        tr_ps, out_res[:, nt, ts(cc, 128)], ident
                    )
                    nc.vector.tensor_copy(
                        out=outT_s[:, cc, ts(nt, 128)], in_=tr_ps
                    )
            # x_outT [f, n] partial = sum_c wpT[c, f] * outT[c, n]
            for fc in range(CC if KPHASE >= 4 else 0):
                xo = p3sb.tile([128, N], F32, tag="xo")
                for n4 in range(4):
                    xo_ps = p3ps.tile([128, 512], F32, tag="xo_ps")
                    for cc in range(VF // 128):
                        nc.tensor.matmul(
                            xo_ps,
                            lhsT=wp_s[:, cc, ts(fc, 128)],
                            rhs=outT_s[:, cc, ts(n4, 512)],
                            start=(cc == 0),
                            stop=(cc == VF // 128 - 1),
                        )
                    nc.vector.tensor_copy(out=xo[:, ts(n4, 512)], in_=xo_ps)
                nc.sync.dma_start(out=xoutT_d[ts(fc, 128), :], in_=xo)

            if KPHASE >= 4:
                nc.gpsimd.collective_compute(
                    "ReduceScatter",
                    mybir.AluOpType.add,
                    replica_groups=[[0, 1, 2, 3], [4, 5, 6, 7]],
                    ins=[xoutT_d.opt()],
                    outs=[rs_d.opt()],
                )
            for cc in range(2 if KPHASE >= 4 else 0):
                xf = p3sb.tile([128, N], F32, tag="xf")
                nc.sync.dma_start(out=xf, in_=rs_d[ts(cc, 128), :])
                nc.vector.tensor_scalar_add(
                    out=xf, in0=xf, scalar1=bp_s[:, cc : cc + 1]
                )
                nc.sync.dma_start(out=xout_d[ts(cc, 128), :], in_=xf)

    nc.compile()
    return nc


def make_in_maps(x, v0, Wqkv, bqkv, Wproj, bproj):
    x = np.asarray(x, np.float32)
    v0 = np.asarray(v0, np.float32)
    Wqkv = np.asarray(Wqkv, np.float32)
    bqkv = np.asarray(bqkv, np.float32)
    Wproj = np.asarray(Wproj, np.float32)
    bproj = np.asarray(bproj, np.float32)

    in_maps = []
    for core in range(NCORES):
        b, hg = divmod(core, HPC)
        hs = hg * HPC          # first head of this core
        r0 = hs * Dh           # first q/k/v row within each C-section

        w_q = (SCALE * Wqkv[r0 : r0 + VF])            # [256, C] pre-scaled
        w_k = Wqkv[C + r0 : C + r0 + VF]
        w_v = Wqkv[2 * C + r0 : 2 * C + r0 + VF]
        b_q = SCALE * bqkv[r0 : r0 + VF]
        b_k = bqkv[C + r0 : C + r0 + VF]
        b_v = bqkv[2 * C + r0 : 2 * C + r0 + VF]

        wqk = np.concatenate([w_q, w_k], 0)           # [512, C]
        bqk = np.concatenate([b_q, b_k], 0)           # [512]

        xT = np.ascontiguousarray(x[b].T).astype(BF16NP)          # [C, N]
        wqkT = np.ascontiguousarray(wqk.T).astype(BF16NP)         # [C, 512]
        wvT = np.ascontiguousarray(w_v.T).astype(BF16NP)          # [C, 256]
        av0 = ALPHA * np.concatenate(
            [v0[b, hs + j] for j in range(HPC)], axis=1
        )                                                          # [N, 256]
        wpT = np.ascontiguousarray(
            Wproj[:, r0 : r0 + VF].T
        ).astype(BF16NP)                                           # [256, C]
        bp = bproj[hg * VF : (hg + 1) * VF]                        # [256]

        in_maps.append({
            "xT": xT.reshape(CC, 128, N),
            "wqkT": wqkT.reshape(CC, 128, QKF),
            "bqk": bqk.reshape(QKF // 128, 128).astype(np.float32),
            "wvT": wvT.reshape(CC, 128, VF),
            "bvb": np.ascontiguousarray(
                np.broadcast_to(b_v, (128, VF))
            ).astype(np.float32),
            "av0": av0.reshape(NT, 128, VF).astype(np.float32),
            "wpT": wpT.reshape(VF // 128, 128, C),
            "bp": bp.reshape(2, 128).astype(np.float32),
        })
    return in_maps


def assemble(results):
    attn = np.empty((B, H, N, N), np.float32)
    x_out = np.empty((B, N, C), np.float32)
    for b in range(B):
        xoutT = np.concatenate(
            [results[b * HPC + i]["xout"] for i in range(HPC)], axis=0
        )  # [C, N]
        x_out[b] = xoutT.T
        for hg in range(HPC):
            hs = hg * HPC
            attn[b, hs : hs + HPC] = results[b * HPC + hg]["attn"]
    return x_out, attn


_NC_CACHE = None


def _get_nc():
    global _NC_CACHE
    if _NC_CACHE is None:
        _NC_CACHE = build_nc()
    return _NC_CACHE


def run(inputs, trace=False):
    nc = _get_nc()
    in_maps = make_in_maps(**inputs)
    res = run_bass_kernel_spmd(
        nc, in_maps, core_ids=list(range(NCORES)), trace=trace
    )
    out = assemble(res.results)
    return out, res


def kernel(**inputs):
    out, _ = run(inputs, trace=False)
    return out
